# revision 1
# baseline (speedup 1.0000x reference)
"""KAN cubic-dict 1D kernel for 8 Trainium2 NeuronCores.

Math: y = id_gain_c*x + bias_c + spline_c(u),  u = 15.5*a_c*x + 15.5*(b_c+1)
clamped so spline_c is the cubic B-spline over per-channel table
T = mix @ alpha_table.T, with the reference's index clamping folded into a
flat extension outside u in [-2, 34].

Device strategy (no per-lane gather exists on TRN2 stock ISA): refit each
channel's clamped cubic spline as a quadratic spline on half-integer knots,
written in two-sided truncated-power form around u_mid = 16:

  spline_c(uc) ~= p2*uc^2 + p1*uc + p0
               + sum_{xi right} eta[c,xi] * relu(uc - xi)^2
               + sum_{xi left } eta[c,xi] * relu(xi - uc)^2

relu chains use the recurrence z_{k+1} = relu(z_k - 1) (one fused
tensor_scalar each), squares run on the Scalar(ACT) engine, and the
per-channel weighted accumulation is one fused scalar_tensor_tensor per
knot. Everything is continuous arithmetic: no floor, no indices, no
gathers. Data parallel: batch 16 -> 2 per core; layout [C=128 partitions,
2*64*64=8192 free] per core.
"""

import numpy as np

import concourse.bacc as bacc
import concourse.mybir as mybir
from concourse import bass_utils
from concourse.tile import TileContext

F32 = mybir.dt.float32
OP = mybir.AluOpType
AF = mybir.ActivationFunctionType

B, C, H, W = 16, 128, 64, 64
K, R, CLAMP = 32, 8, 1.5
NCORES = 8
BPC = B // NCORES              # batches per core
NFREE = BPC * H * W            # 8192
NT = 2048                      # free-dim tile
NCHUNK = NFREE // NT

U_LO, U_HI = -2.0, 34.0
U_MID = 16.0
# half-integer knots strictly inside (U_LO, U_HI)
KNOTS_R = [16.5 + i for i in range(18)]          # 16.5 .. 33.5
KNOTS_L = [15.5 - i for i in range(18)]          # 15.5 .. -1.5
NKNOT = len(KNOTS_R) + len(KNOTS_L)

# const layout (columns of the [128, NCONST] constant tensor)
COL_S, COL_O, COL_P2, COL_P1, COL_P0, COL_G = 0, 1, 2, 3, 4, 5
COL_ETA = 6                                       # 6 .. 6+36
NCONST = COL_ETA + NKNOT


def _spline_exact(T, u):
    """Exact clamped cubic B-spline, vectorized over channels.
    T: (C, K) float64; u: (G,) in [-2, 34]. Returns (C, G)."""
    uc = np.clip(u, U_LO, U_HI)
    i = np.clip(np.floor(uc).astype(np.int64), -2, 33)
    t = uc - i
    t2 = t * t
    t3 = t2 * t
    bs = [(1 - 3 * t + 3 * t2 - t3) / 6, (4 - 6 * t2 + 3 * t3) / 6,
          (1 + 3 * t + 3 * t2 - 3 * t3) / 6, t3 / 6]
    out = np.zeros((T.shape[0], u.shape[0]))
    for j, bj in enumerate(bs):
        idx = np.clip(i - 1 + j, 0, K - 1)
        out += T[:, idx] * bj[None, :]
    return out


def _b2(z):
    z = np.abs(z)
    return np.where(z < 0.5, 0.75 - z * z,
                    np.where(z < 1.5, 0.5 * (1.5 - z) ** 2, 0.0))


def _b2pp(z):
    z = np.abs(z)
    return np.where(z < 0.5, -2.0, np.where(z < 1.5, 1.0, 0.0))


def _host_precompute(a, b, alpha_table, mix, id_gain, bias):
    """Returns consts (C, NCONST) float32."""
    T = mix.astype(np.float64) @ alpha_table.astype(np.float64).T  # (C, K)

    centers = np.arange(-3, 37, dtype=np.float64)                  # 40 B2 centers
    grid = np.linspace(U_LO, U_HI, 4001)
    V = _b2(grid[:, None] - centers[None, :])                      # (G, M)
    Y = _spline_exact(T, grid)                                     # (C, G)
    Wc, *_ = np.linalg.lstsq(V, Y.T, rcond=None)                   # (M, C)

    # piecewise-constant s'' of the fitted quadratic spline, on intervals
    # between half-integer knots; sample at interval midpoints
    knots_all = np.arange(-2.0, 34.5, 0.5)  # not needed; use analytic jumps
    def spp(u):   # (C,) second derivative at u (not at a knot)
        return (_b2pp(u - centers[None, :]) * Wc.T).sum(axis=1)

    # mid quadratic: exact quadratic piece of s on [15.5, 16.5]
    # fit through 3 interior points
    up = np.array([15.6, 16.0, 16.4])
    Vp = _b2(up[:, None] - centers[None, :])
    yp = (Vp @ Wc)                            # (3, C)
    A3 = np.stack([up * up, up, np.ones(3)], axis=1)
    P = np.linalg.solve(A3, yp)               # (3, C): rows p2, p1, p0
    p2, p1, p0 = P[0], P[1], P[2]

    etas = np.zeros((C, NKNOT))
    for j, xi in enumerate(KNOTS_R):
        etas[:, j] = (spp(xi + 0.25) - spp(xi - 0.25)) / 2.0
    for j, xi in enumerate(KNOTS_L):
        etas[:, len(KNOTS_R) + j] = (spp(xi - 0.25) - spp(xi + 0.25)) / 2.0

    consts = np.zeros((C, NCONST), dtype=np.float64)
    consts[:, COL_S] = 15.5 * a
    consts[:, COL_O] = 15.5 * (b + 1.0)
    consts[:, COL_P2] = p2
    consts[:, COL_P1] = p1
    consts[:, COL_P0] = p0 + bias
    consts[:, COL_G] = id_gain
    consts[:, COL_ETA:COL_ETA + NKNOT] = etas
    return consts.astype(np.float32)


def host_eval(x_cn, consts):
    """fp32 simulation of the device op order. x_cn: (C, N) f32."""
    s = consts[:, COL_S:COL_S + 1]
    o = consts[:, COL_O:COL_O + 1]
    u = (x_cn * s + o).astype(np.float32)
    uc = np.clip(u, U_LO, U_HI).astype(np.float32)
    h = (uc * consts[:, COL_P2:COL_P2 + 1] + consts[:, COL_P1:COL_P1 + 1]).astype(np.float32)
    acc = (h * uc).astype(np.float32)
    z = np.maximum(uc - KNOTS_R[0], 0.0).astype(np.float32)
    for j, xi in enumerate(KNOTS_R):
        if j > 0:
            z = np.maximum(z - 1.0, 0.0).astype(np.float32)
        q = (z * z).astype(np.float32)
        acc = (q * consts[:, COL_ETA + j:COL_ETA + j + 1] + acc).astype(np.float32)
    z = np.maximum(KNOTS_L[0] - uc, 0.0).astype(np.float32)
    for j, xi in enumerate(KNOTS_L):
        if j > 0:
            z = np.maximum(z - 1.0, 0.0).astype(np.float32)
        q = (z * z).astype(np.float32)
        jj = len(KNOTS_R) + j
        acc = (q * consts[:, COL_ETA + jj:COL_ETA + jj + 1] + acc).astype(np.float32)
    y = (x_cn * consts[:, COL_G:COL_G + 1] + acc).astype(np.float32)
    y = (y + consts[:, COL_P0:COL_P0 + 1]).astype(np.float32)
    return y


def _build_program():
    nc = bacc.Bacc("TRN2", target_bir_lowering=False)
    xs = nc.dram_tensor("xs", (C, NFREE), F32, kind="ExternalInput")
    cst = nc.dram_tensor("cst", (C, NCONST), F32, kind="ExternalInput")
    ys = nc.dram_tensor("ys", (C, NFREE), F32, kind="ExternalOutput")

    with TileContext(nc) as tc:
        with (
            tc.tile_pool(name="cpool", bufs=1) as cpool,
            tc.tile_pool(name="io", bufs=2) as io,
            tc.tile_pool(name="wk", bufs=2) as wk,
            tc.tile_pool(name="ac", bufs=2) as ac,
        ):
            ct = cpool.tile([C, NCONST], F32, tag="cst")
            nc.sync.dma_start(ct[:], cst[:])

            def col(j):
                return ct[:, j:j + 1]

            for ci in range(NCHUNK):
                sl = slice(ci * NT, (ci + 1) * NT)
                xt = io.tile([C, NT], F32, tag="x")
                nc.sync.dma_start(xt[:], xs[:, sl])

                u = wk.tile([C, NT], F32, tag="u")
                nc.scalar.activation(u[:], xt[:], AF.Identity,
                                     bias=col(COL_O), scale=col(COL_S))
                uc = wk.tile([C, NT], F32, tag="uc")
                nc.vector.tensor_scalar(uc[:], u[:], U_HI, U_LO,
                                        op0=OP.min, op1=OP.max)

                h = wk.tile([C, NT], F32, tag="h")
                nc.vector.tensor_scalar(h[:], uc[:], col(COL_P2), col(COL_P1),
                                        op0=OP.mult, op1=OP.add)
                acc = ac.tile([C, NT], F32, tag="acc")
                nc.vector.tensor_tensor(acc[:], h[:], uc[:], op=OP.mult)

                for side, knots in (("R", KNOTS_R), ("L", KNOTS_L)):
                    zprev = None
                    for j, xi in enumerate(knots):
                        z = wk.tile([C, NT], F32, tag=f"z{side}")
                        if j == 0:
                            if side == "R":
                                nc.vector.tensor_scalar(
                                    z[:], uc[:], -xi, 0.0,
                                    op0=OP.add, op1=OP.max)
                            else:
                                zp = wk.tile([C, NT], F32, tag="zLp")
                                nc.vector.tensor_scalar(
                                    zp[:], uc[:], -1.0, xi,
                                    op0=OP.mult, op1=OP.add)
                                nc.vector.tensor_scalar(
                                    z[:], zp[:], 0.0, None, op0=OP.max)
                        else:
                            eng = nc.gpsimd if (j % 4) else nc.vector
                            eng.tensor_scalar(z[:], zprev[:], -1.0, 0.0,
                                              op0=OP.add, op1=OP.max)
                        q = wk.tile([C, NT], F32, tag=f"q{side}")
                        nc.scalar.activation(q[:], z[:], AF.Square)
                        jj = j if side == "R" else len(KNOTS_R) + j
                        nc.vector.scalar_tensor_tensor(
                            acc[:], q[:], col(COL_ETA + jj), acc[:],
                            op0=OP.mult, op1=OP.add)
                        zprev = z

                y1 = io.tile([C, NT], F32, tag="y")
                nc.vector.scalar_tensor_tensor(
                    y1[:], xt[:], col(COL_G), acc[:],
                    op0=OP.mult, op1=OP.add)
                nc.vector.tensor_scalar(y1[:], y1[:], col(COL_P0), None,
                                        op0=OP.add)
                nc.sync.dma_start(ys[:, sl], y1[:])
    nc.finalize()
    return nc


_CACHED = {}


def kernel(x, a, b, alpha_table, mix, id_gain, bias):
    x = np.ascontiguousarray(np.asarray(x, dtype=np.float32))
    consts = _host_precompute(
        np.asarray(a, np.float64), np.asarray(b, np.float64),
        np.asarray(alpha_table, np.float64), np.asarray(mix, np.float64),
        np.asarray(id_gain, np.float64), np.asarray(bias, np.float64))

    if "nc" not in _CACHED:
        _CACHED["nc"] = _build_program()
    nc = _CACHED["nc"]

    in_maps = []
    for g in range(NCORES):
        xg = x[g * BPC:(g + 1) * BPC]                    # (BPC, C, H, W)
        x_cn = np.ascontiguousarray(
            xg.transpose(1, 0, 2, 3).reshape(C, NFREE))
        in_maps.append({"xs": x_cn, "cst": consts})

    res = bass_utils.run_bass_kernel_spmd(nc, in_maps, list(range(NCORES)))
    y = np.empty((B, C, H, W), dtype=np.float32)
    for g in range(NCORES):
        y_cn = res.results[g]["ys"].reshape(C, BPC, H, W)
        y[g * BPC:(g + 1) * BPC] = y_cn.transpose(1, 0, 2, 3)
    return y

